# revision 1
# baseline (speedup 1.0000x reference)
"""Data-parallel Trainium kernel for nn_AttentionSACModel.

Shards the batch dim of obs across the 8 NeuronCores (pure data parallel,
params replicated), computes the attention-SAC forward pass on each core,
gathers to the full [32768, 4] output.
"""
import numpy as np
import jax
import jax.numpy as jnp
from functools import partial

OWN_DIM = 7
INTR_DIM = 5
N_HEADS = 3
HEAD_DIM = 5
N_INTR = 256
BATCH = 32768
H1, H2 = 256, 256
ACTION_DIM = 2
IN_DIM = OWN_DIM + N_HEADS * HEAD_DIM  # 22
SCALE = np.sqrt(np.float32(HEAD_DIM)).astype(np.float32)
LN_EPS = 1e-5
N_CORES = 8

PARAM_NAMES = [
    "Wq", "bq", "Wk", "bk", "Wv", "bv", "v_att", "temperature",
    "ln_g", "ln_b", "W1", "b1", "W2", "b2", "Wf", "bf", "log_std",
]


def _forward(obs, p):
    own = obs[:, :OWN_DIM]
    intr = obs[:, OWN_DIM:OWN_DIM + N_INTR * INTR_DIM].reshape(-1, N_INTR, INTR_DIM)

    q = jnp.einsum("bo,hod->bhd", own, p["Wq"]) + p["bq"]
    k = jnp.einsum("bni,hid->bhnd", intr, p["Wk"]) + p["bk"][None, :, None, :]

    energy = jnp.tanh(q[:, :, None, :] + k)
    scores = jnp.einsum("bhnd,hd->bhn", energy, p["v_att"])
    scores = scores / SCALE * jnp.abs(p["temperature"][0])

    is_padding = jnp.sum(jnp.abs(intr), axis=2) < 1e-6
    scores = jnp.where(is_padding[:, None, :], -jnp.inf, scores)
    alpha = jax.nn.softmax(scores, axis=-1)
    alpha = jnp.nan_to_num(alpha, nan=0.0)

    # context = (alpha-weighted sum of intr) @ Wv + bv * sum(alpha)
    wsum = jnp.einsum("bhn,bni->bhi", alpha, intr)
    asum = jnp.sum(alpha, axis=-1)  # [B, H]
    context = jnp.einsum("bhi,hid->bhd", wsum, p["Wv"]) + asum[:, :, None] * p["bv"][None, :, :]
    context = context.reshape(context.shape[0], N_HEADS * HEAD_DIM)

    x = jnp.concatenate([own, context], axis=1)
    mu = jnp.mean(x, axis=-1, keepdims=True)
    var = jnp.mean(jnp.square(x - mu), axis=-1, keepdims=True)
    x = (x - mu) * jax.lax.rsqrt(var + LN_EPS) * p["ln_g"] + p["ln_b"]

    x = jax.nn.leaky_relu(x @ p["W1"] + p["b1"], negative_slope=0.2)
    x = jax.nn.leaky_relu(x @ p["W2"] + p["b2"], negative_slope=0.2)
    out = x @ p["Wf"] + p["bf"]

    log_std_exp = jnp.broadcast_to(p["log_std"][None, :], (out.shape[0], ACTION_DIM))
    return jnp.concatenate([out, log_std_exp], axis=1)


_pmapped = None


def _get_pmapped():
    global _pmapped
    if _pmapped is None:
        _pmapped = jax.pmap(_forward, in_axes=(0, None), devices=jax.devices()[:N_CORES])
    return _pmapped


def kernel(**inputs) -> np.ndarray:
    obs = np.asarray(inputs["obs"], dtype=np.float32)
    p = {n: jnp.asarray(np.asarray(inputs[n], dtype=np.float32)) for n in PARAM_NAMES}
    obs_sharded = obs.reshape(N_CORES, BATCH // N_CORES, obs.shape[1])
    out = _get_pmapped()(obs_sharded, p)
    out = np.asarray(out).reshape(BATCH, ACTION_DIM * 2)
    return out
